# revision 1
# baseline (speedup 1.0000x reference)
"""8-core Trainium2 Bass kernel v2 for nn_Attention_89489938579587.

reference: qkv = x @ w_attn.T; split q,k,v per 16 heads (HD=128); RoPE
(interleaved pairs); non-causal SDPA; y @ w_proj.T.  B=4, T=2048, D=2048.

Sharding v2: core i -> (batch b=i//2, head-half hh=i%2).  Each core computes
QKV for its 8 heads over ALL 2048 tokens, RoPE, SDPA, and a PARTIAL output
projection (contraction over its 1024 head-dims) in f-major layout
[2048 f, 2048 t].  Host adds the two partials per batch and transposes.
This is the ideal 1/8 compute share (25.8 GMAC/core); no collectives.

All tensors stored bf16 (halves SBUF/DMA, doubles DVE rate); matmuls bf16
at full PE rate; PSUM f32.  Single fused TileContext: QKV(h+1) matmuls are
interleaved into SDPA(h) program order so the PE never waits on softmax.
V is computed token-major directly (x chunks stationary, 4-head weight
slabs moving) so no transpose is needed.  Softmax denominators are reduced
via a ones-vector matmul, batched per-head reciprocal on DVE, and
partition-broadcast on GpSimd.

Self-contained: builds the Bass program on first call, runs via
run_bass_kernel_spmd on cores 0-7.
"""

import numpy as np
from contextlib import ExitStack

import concourse.bass as bass
import concourse.tile as tile
from concourse import mybir
from concourse.bass import ts

import bass_rust
import ml_dtypes

# ---------------------------------------------------------------------------
# Toolchain workarounds (same as baseline): walrus rejects >1 sem wait per
# instruction; split extras onto same-engine nops; patch tile drain.
# ---------------------------------------------------------------------------


def _split_multi_waits(nc, max_waits=1):
    n = 0
    for fn in nc.m.functions:
        for blk in fn.blocks:
            insts = blk.instructions
            i = 0
            while i < len(insts):
                inst = insts[i]
                si = inst.sync_info
                waits = list(si.on_wait) if (si is not None and si.on_wait) else []
                if len(waits) > max_waits:
                    si.on_wait = waits[:max_waits]
                    extra = waits[max_waits:]
                    for j in range(0, len(extra), max_waits):
                        nop = mybir.InstNoOp(
                            name=nc.get_next_instruction_name(), ins=[], outs=[])
                        nop.engine = inst.engine
                        nop.sync_info = bass_rust.SyncInfo(
                            on_wait=extra[j:j + max_waits], on_update=[])
                        nc.register_instruction(nop, overwrite=True)
                        insts.insert(i, nop)
                        i += 1
                        n += 1
                i += 1
    return n


def _patched_drain_and_barrier(self, tick_clock, wait_clock):
    from concourse.vector_clock import ScopedClock
    nc = self.nc
    probe = nc.sync.nop()
    wait_clock.add_sem_waits(probe.ins, ScopedClock({None: tick_clock.global_clock}))
    si = probe.ins.sync_info
    waits = list(si.on_wait or []) if si is not None else []
    if len(waits) > 1:
        si.on_wait = [waits[0]]
        for w in waits[1:]:
            nop = nc.sync.nop()
            nsi = nop.ins.sync_info
            if nsi is None:
                nop.ins.sync_info = bass_rust.SyncInfo(on_wait=[w], on_update=[])
            else:
                nsi.on_wait = [w]
    nc.sync.drain()
    nc.all_engine_barrier()
    assert self.sems is not None
    popped = nc._tile_sem_poison_stack.pop()
    assert popped is self._sem_poison
    nc.clear_and_free_semaphores(list(self.sems.allocated().values()))
    nc.all_engine_barrier()


_patched = False


def _apply_patches():
    global _patched
    if not _patched:
        tile.TileContext._drain_and_barrier = _patched_drain_and_barrier
        _patched = True


# ---------------------------------------------------------------------------
# Problem constants
# ---------------------------------------------------------------------------
BF16 = mybir.dt.bfloat16
F32 = mybir.dt.float32
F32R = mybir.dt.float32r
EXP = mybir.ActivationFunctionType.Exp

B, T, D, H, HD = 4, 2048, 2048, 16, 128
CC = D // 128            # 16 contraction chunks
NH = 8                   # heads per core
KC = T // 128            # 16 key chunks
SCALE = 1.0 / float(np.sqrt(HD))
N_CORES = 8


def build_nc(n_cores=N_CORES):
    _apply_patches()
    nc = bass.Bass("TRN2", target_bir_lowering=False, debug=False,
                   num_devices=n_cores)
    xT = nc.dram_tensor("xT", [CC, 128, T], BF16, kind="ExternalInput").ap()
    wqs = nc.dram_tensor("wqs", [NH, 128, CC * 128], BF16, kind="ExternalInput").ap()
    wks = nc.dram_tensor("wks", [NH, 128, CC * 128], BF16, kind="ExternalInput").ap()
    # V weights: [group, cc-half, 128p, 8cc * (4h*128f)]
    wvs = nc.dram_tensor("wvs", [2, 2, 128, 8 * 512], BF16, kind="ExternalInput").ap()
    # out-proj: [128p(hd), fc, hc, 128f]
    wps = nc.dram_tensor("wps", [128, CC, NH, 128], BF16, kind="ExternalInput").ap()
    cs2 = nc.dram_tensor("cs2", [128, T], BF16, kind="ExternalInput").ap()
    sn2 = nc.dram_tensor("sn2", [128, T], BF16, kind="ExternalInput").ap()
    onesd = nc.dram_tensor("onesd", [128, 512], F32R, kind="ExternalInput").ap()
    # broadcast selectors: sels[p, v, :] = 1.0 iff p == 32*v
    seld = nc.dram_tensor("seld", [64, 2, 128], BF16, kind="ExternalInput").ap()
    onescd = nc.dram_tensor("onescd", [128, 1], BF16, kind="ExternalInput").ap()
    # f-major partial output [2048 f, 2048 t]
    out = nc.dram_tensor("out", [D, T], BF16, kind="ExternalOutput").ap()

    with tile.TileContext(nc) as tc, ExitStack() as octx:
        cs_pool = octx.enter_context(tc.tile_pool(name="cs", bufs=1))
        csk = cs_pool.tile([128, T], BF16, tag="csk")
        snk = cs_pool.tile([128, T], BF16, tag="snk")
        onesb = cs_pool.tile([128, 512], F32R, tag="onesb")
        ones128 = cs_pool.tile([128, 1], F32R, tag="ones128")
        ones1 = cs_pool.tile([1, 128], F32R, tag="ones1")
        sels = cs_pool.tile([64, 2, 128], BF16, tag="sels")
        onesc = cs_pool.tile([128, 1], BF16, tag="onesc")

        def load_tables():
            # issued AFTER the wv/x loads so the first V matmuls aren't
            # queued behind 10 MB of table traffic; nothing needs these
            # for the first ~15 us
            nc.sync.dma_start(csk[:], cs2[:])
            nc.sync.dma_start(snk[:], sn2[:])
            nc.sync.dma_start(onesb[:], onesd[:])
            nc.sync.dma_start(ones128[:], onesd[:, 0:1])
            nc.sync.dma_start(ones1[:], onesd[0:1, 0:128])
            nc.sync.dma_start(sels[:], seld[:])
            nc.sync.dma_start(onesc[:], onescd[:])

        oT_pool = octx.enter_context(tc.tile_pool(name="oT", bufs=1))
        oT = oT_pool.tile([128, NH, T], BF16, tag="oT")

        with ExitStack() as p1:
            x_pool = p1.enter_context(tc.tile_pool(name="x", bufs=1))
            wqk_pool = p1.enter_context(tc.tile_pool(name="wqk", bufs=3))
            wv_pool = p1.enter_context(tc.tile_pool(name="wv", bufs=2))
            vg_pool = p1.enter_context(tc.tile_pool(name="vg", bufs=1))
            q_pool = p1.enter_context(tc.tile_pool(name="q", bufs=2))
            k_pool = p1.enter_context(tc.tile_pool(name="k", bufs=2))
            rp_pool = p1.enter_context(tc.tile_pool(name="rp", bufs=2))
            eT_pool = p1.enter_context(tc.tile_pool(name="eT", bufs=5))
            es_pool = p1.enter_context(tc.tile_pool(name="es", bufs=2))
            ob_pool = p1.enter_context(tc.tile_pool(name="ob", bufs=6))
            r_pool = p1.enter_context(tc.tile_pool(name="r", bufs=4))
            ps_qkv = p1.enter_context(tc.tile_pool(name="psqkv", bufs=2, space="PSUM"))
            ps_s = p1.enter_context(tc.tile_pool(name="pss", bufs=3, space="PSUM"))
            ps_o = p1.enter_context(tc.tile_pool(name="pso", bufs=1, space="PSUM"))
            ps_aux = p1.enter_context(tc.tile_pool(name="psaux", bufs=1, space="PSUM"))
            ps_bc = p1.enter_context(tc.tile_pool(name="psbc", bufs=1, space="PSUM"))

            # x resident: 16 tiles [128, 2048] bf16 (loaded in bootstrap,
            # after the V weight slabs)
            xs = [None] * CC

            # per-core state holders
            q_sb = [None] * NH
            k_sb = [None] * NH
            v_sb = [None] * 2   # per group

            w_hold = {"q": None, "k": None}
            sf_hold = [None]

            def emit_proj_half(kind, h, tpair, half):
                """One 16-matmul unit: 512 tokens of a Q/K projection.  After
                the second half of a token-pair, the RoPE chain is emitted."""
                if tpair == 0 and half == 0:
                    wsl = wqk_pool.tile([128, CC, 128], BF16, tag="wqk")
                    nc.sync.dma_start(wsl[:], (wqs if kind == "q" else wks)[h])
                    w_hold[kind] = wsl
                else:
                    wsl = w_hold[kind]
                toff = tpair * 1024
                if half == 0:
                    sf_hold[0] = rp_pool.tile([128, 1024], BF16, tag="sf",
                                              name=f"sf_{kind}{h}_{tpair}")
                sf = sf_hold[0]
                ps = ps_qkv.tile([128, 512], F32, tag="psqkv")
                for cc in range(CC):
                    nc.tensor.matmul(
                        ps[:], wsl[:, cc, :],
                        xs[cc][:, toff + half * 512: toff + (half + 1) * 512],
                        start=(cc == 0), stop=(cc == CC - 1))
                nc.scalar.copy(sf[:, ts(half, 512)], ps[:])
                if half == 0:
                    return
                sw = rp_pool.tile([128, 1024], BF16, tag="sw")
                nc.sync.dma_start(sw[0:64, :], sf[64:128, :])
                nc.sync.dma_start(sw[64:128, :], sf[0:64, :])
                nc.vector.tensor_mul(sf[:], sf[:], csk[:, toff:toff + 1024])
                nc.vector.tensor_mul(sw[:], sw[:], snk[:, toff:toff + 1024])
                dst = q_sb[h] if kind == "q" else k_sb[h]
                nc.vector.tensor_add(dst[:, toff:toff + 1024], sf[:], sw[:])

            def producer_units(hn):
                """Generator of head-hn QKV producer units (8 per head)."""
                q_sb[hn] = q_pool.tile([128, T], BF16, tag="q", name=f"qh{hn}")
                k_sb[hn] = k_pool.tile([128, T], BF16, tag="k", name=f"kh{hn}")
                for kind in ("q", "k"):
                    for tpair in range(2):
                        for half in range(2):
                            yield (kind, hn, tpair, half)

            def emit_v_chunk(g, tch_pair):
                """V for head-group g, two token chunks (2*128 tokens)."""
                for u in range(2):
                    tch = tch_pair * 2 + u
                    ps = ps_qkv.tile([128, 512], F32, tag="psqkv")
                    for cc in range(CC):
                        wv_ap = v_w[g][cc // 8][:, (cc % 8) * 512:(cc % 8 + 1) * 512]
                        nc.tensor.matmul(ps[:], xs[cc][:, ts(tch, 128)], wv_ap,
                                         start=(cc == 0), stop=(cc == CC - 1))
                    if tch % 2 == 0:
                        nc.scalar.copy(v_sb[g][:, tch, :], ps[:])
                    else:
                        nc.vector.tensor_copy(v_sb[g][:, tch, :], ps[:])

            # V weight slabs for both groups, halves double-buffered
            v_w = [[None, None], [None, None]]

            def load_vw(g, half):
                w_ = wv_pool.tile([128, 8 * 512], BF16, tag="wv")
                nc.sync.dma_start(w_[:], wvs[g, half])
                v_w[g][half] = w_

            # ---------------- bootstrap ----------------
            load_vw(0, 0)
            load_vw(0, 1)
            for cc in range(CC):
                t_ = x_pool.tile([128, T], BF16, tag=f"x{cc}", name=f"x{cc}")
                nc.sync.dma_start(t_[:], xT[cc])
                xs[cc] = t_
            load_tables()
            v_sb[0] = vg_pool.tile([128, KC, 512], BF16, tag="vg", name="vg0")
            for tp in range(8):
                emit_v_chunk(0, tp)
            for unit in producer_units(0):
                emit_proj_half(*unit)

            # deferred per-(h, qt) normalizations: bc matmul + DVE multiply,
            # consumed inside the NEXT head's kc loop so the PE never waits.
            pending_norms = []

            def emit_norm(h, qt, o_t, r_t):
                bc = ps_bc.tile([128, 512], F32, tag="psbc")
                nc.tensor.matmul(bc[:], sels[0:1, 0, :], r_t[:],
                                 start=True, stop=True)
                with nc.allow_low_precision(reason="bf16 attn out"):
                    nc.vector.tensor_mul(oT[:, h, ts(qt, 512)], o_t[:], bc[:])

            def pop_norm(lag=2):
                # keep >= `lag` entries queued so the bc matmul never waits
                # on a reciprocal still draining through the DVE queue
                if len(pending_norms) > lag:
                    emit_norm(*pending_norms.pop(0))

            # ---------------- main loop: SDPA per head ----------------
            for h in range(NH):
                g, j = h // 4, h % 4
                prod = producer_units(h + 1) if h + 1 < NH else iter(())
                for qt in range(4):
                    aux = ps_aux.tile([128, 512], F32, tag="psaux",
                                      name=f"aux{h}_{qt}")
                    qsl = q_sb[h][:, ts(qt, 512)]
                    esum = es_pool.tile([128, 512], BF16, tag="es")
                    o_ps = ps_o.tile([128, 512], F32, tag="pso")
                    eTs = [None] * KC

                    def pv(kc):
                        nc.tensor.matmul(
                            o_ps[:], v_sb[g][:, kc, ts(j, 128)], eTs[kc][:],
                            start=(kc == 0), stop=(kc == KC - 1))

                    for kc in range(KC):
                        s_ps = ps_s.tile([128, 512], F32, tag="pss")
                        nc.tensor.matmul(s_ps[:], k_sb[h][:, ts(kc, 128)], qsl,
                                         start=True, stop=True)
                        eT = eT_pool.tile([128, 512], BF16, tag="eT")
                        nc.scalar.activation(eT[:], s_ps[:], EXP, scale=SCALE)
                        eTs[kc] = eT
                        # bf16 chunk-sum (magnitude ~30; the 2048-wide key
                        # reduction happens exactly in f32 PSUM below)
                        with nc.allow_low_precision(reason="bf16 chunk sum"):
                            if kc == 0:
                                nc.vector.tensor_copy(esum[:], eT[:])
                            else:
                                nc.vector.tensor_add(esum[:], esum[:], eT[:])
                        if kc >= 2:
                            pv(kc - 2)
                        if kc == 2:
                            pop_norm()
                        if kc in (5, 11):
                            unit = next(prod, None)
                            if unit is not None:
                                emit_proj_half(*unit)
                    pv(KC - 2)
                    pv(KC - 1)
                    # denominator: [1, 512] at psum row 0
                    nc.tensor.matmul(aux[0:1, :], onesc[:], esum[:],
                                     start=True, stop=True)
                    o_sb = ob_pool.tile([128, 512], BF16, tag="ob")
                    nc.scalar.copy(o_sb[:], o_ps[:])
                    r_sb = r_pool.tile([1, 512], BF16, tag="r",
                                       name=f"r{h}_{qt}")
                    with nc.allow_low_precision(reason="bf16 softmax denom"):
                        nc.vector.reciprocal(r_sb[:], aux[0:1, :])
                    pending_norms.append((h, qt, o_sb, r_sb))

                # V(g1) block between head 3 and head 4: v_sb[0] reads are
                # all emitted by now, so the single vg buffer can recycle.
                if h == 3:
                    load_vw(1, 0)
                    load_vw(1, 1)
                    v_sb[1] = vg_pool.tile([128, KC, 512], BF16, tag="vg", name="vg1")
                    for tp in range(8):
                        emit_v_chunk(1, tp)

            # flush the remaining deferred normalizations
            while pending_norms:
                emit_norm(*pending_norms.pop(0))

        # ---------------- phase 3: partial output projection ----------------
        with ExitStack() as p3:
            wp_pool = p3.enter_context(tc.tile_pool(name="wp", bufs=1))
            oe_pool = p3.enter_context(tc.tile_pool(name="oe", bufs=4))
            ps3 = p3.enter_context(tc.tile_pool(name="ps3", bufs=4, space="PSUM"))

            for fc in range(CC):
                wp_fc = wp_pool.tile([128, NH, 128], BF16, tag=f"wp{fc}")
                nc.sync.dma_start(wp_fc[:], wps[:, fc])
                pss = [ps3.tile([128, 512], F32, tag="ps3", name=f"ps3_{fc}_{i}") for i in range(4)]
                for hc in range(NH):
                    for tq in range(4):
                        nc.tensor.matmul(pss[tq][:], wp_fc[:, hc, :],
                                         oT[:, hc, ts(tq, 512)],
                                         start=(hc == 0), stop=(hc == NH - 1))
                for tq in range(4):
                    oe = oe_pool.tile([128, 512], BF16, tag="oe")
                    if tq % 2 == 0:
                        nc.scalar.copy(oe[:], pss[tq][:])
                    else:
                        nc.vector.tensor_copy(oe[:], pss[tq][:])
                    nc.sync.dma_start(out[ts(fc, 128), ts(tq, 512)], oe[:])

    _split_multi_waits(nc)
    return nc


# ---------------------------------------------------------------------------
# host-side prep / assembly
# ---------------------------------------------------------------------------


def _to_bf16(a):
    return np.ascontiguousarray(a.astype(ml_dtypes.bfloat16))


def prep_inputs(x, w_attn, w_proj):
    x = np.asarray(x, dtype=np.float32)
    w_attn = np.asarray(w_attn, dtype=np.float32)
    w_proj = np.asarray(w_proj, dtype=np.float32)

    perm = np.concatenate([np.arange(0, HD, 2), np.arange(1, HD, 2)])

    inv = 1.0 / (10000.0 ** (np.arange(0, HD, 2, dtype=np.float64) / HD))
    fr = np.outer(np.arange(T, dtype=np.float64), inv)
    cos = np.cos(fr).T
    sin = np.sin(fr).T
    cs2 = _to_bf16(np.concatenate([cos, cos], 0))
    sn2 = _to_bf16(np.concatenate([-sin, sin], 0))
    onesd = np.ones((128, 512), dtype=np.float32)
    seld = np.zeros((64, 2, 128), dtype=np.float32)
    seld[0, 0, :] = 1.0
    seld[32, 1, :] = 1.0
    seld = _to_bf16(seld)
    onescd = _to_bf16(np.ones((128, 1), dtype=np.float32))

    # per head-half weight slabs (shared across batches)
    half_slabs = []
    for hh in range(2):
        heads = range(hh * NH, (hh + 1) * NH)
        # wq/wk: [NH, 128p(c within cc), CC*128f] with rope perm on f
        wq_sl = np.empty((NH, 128, CC * 128), dtype=np.float32)
        wk_sl = np.empty((NH, 128, CC * 128), dtype=np.float32)
        for jj, h in enumerate(heads):
            wq_h = w_attn[h * HD:(h + 1) * HD][perm, :]        # [128f, 2048c]
            wk_h = w_attn[D + h * HD:D + (h + 1) * HD][perm, :]
            # slab[p, cc, f] = w[f, cc*128+p]
            wq_sl[jj] = wq_h.T.reshape(CC, 128, 128).transpose(1, 0, 2).reshape(128, -1)
            wk_sl[jj] = wk_h.T.reshape(CC, 128, 128).transpose(1, 0, 2).reshape(128, -1)
        # wv: [2 groups, 2 halves, 128p, 8cc*(4h*128)]
        wv_sl = np.empty((2, 2, 128, 8 * 512), dtype=np.float32)
        for g in range(2):
            hv = w_attn[2 * D + (hh * NH + g * 4) * HD:
                        2 * D + (hh * NH + (g + 1) * 4) * HD]  # [512f, 2048c]
            # [cc, p, f] -> [half, 128p, 8cc, 512f]
            arr = hv.T.reshape(CC, 128, 512)
            for half in range(2):
                wv_sl[g, half] = (arr[half * 8:(half + 1) * 8]
                                  .transpose(1, 0, 2).reshape(128, -1))
        # wp: [128p(hd within hc), fc, hc, 128f]
        #   value = w_proj[fc*128+f, hh*1024 + hc*128 + p]
        wp_cols = w_proj[:, hh * NH * HD:(hh + 1) * NH * HD]  # [2048f, 1024hd]
        wp_sl = (wp_cols.T.reshape(NH, 128, CC, 128)
                 .transpose(1, 2, 0, 3))                       # [128p, fc, hc, f]
        half_slabs.append((_to_bf16(wq_sl), _to_bf16(wk_sl), _to_bf16(wv_sl),
                           _to_bf16(np.ascontiguousarray(wp_sl))))

    xTs = []
    for b in range(B):
        xT = x[b].T.reshape(CC, 128, T)
        xTs.append(_to_bf16(xT))

    in_maps = []
    for i in range(N_CORES):
        b, hh = i // 2, i % 2
        wq_sl, wk_sl, wv_sl, wp_sl = half_slabs[hh]
        in_maps.append({
            "xT": xTs[b],
            "wqs": wq_sl, "wks": wk_sl, "wvs": wv_sl, "wps": wp_sl,
            "cs2": cs2, "sn2": sn2, "onesd": onesd, "seld": seld,
            "onescd": onescd,
        })
    return in_maps


def assemble(results):
    out = np.empty((B, T, D), dtype=np.float32)
    for b in range(B):
        p0 = results[2 * b]["out"].astype(np.float32)
        p1 = results[2 * b + 1]["out"].astype(np.float32)
        out[b] = (p0 + p1).T
    return out


_nc_cache = None


def _get_nc():
    global _nc_cache
    if _nc_cache is None:
        _nc_cache = build_nc()
    return _nc_cache


def kernel(x, w_attn, w_proj):
    from concourse.bass_utils import run_bass_kernel_spmd
    nc = _get_nc()
    in_maps = prep_inputs(x, w_attn, w_proj)
    res = run_bass_kernel_spmd(nc, in_maps, list(range(N_CORES)))
    return assemble(res.results)


def run_profiled(x, w_attn, w_proj, trace_cores=None):
    """Like kernel() but with NTFF profiling; returns BassKernelResults."""
    from concourse.bass_utils import run_bass_kernel_spmd
    import sys as _sys, types as _types
    try:
        import antenv
        if "antenv.axon_hooks" not in _sys.modules:
            mod = _types.ModuleType("antenv.axon_hooks")
            _h = [None]
            mod.set_axon_ntff_profile_hook = lambda h: _h.__setitem__(0, h)
            mod.get_axon_ntff_profile_hook = lambda: _h[0]
            _sys.modules["antenv.axon_hooks"] = mod
            antenv.axon_hooks = mod
            from trn_agent_boot.trn_boot import _ntff_profile_via_ctypes
            mod.set_axon_ntff_profile_hook(
                _ntff_profile_via_ctypes('/opt/axon/libaxon_pjrt.so'))
    except Exception as e:  # profiling is best-effort
        print("profile hook setup failed:", e)
    nc = _get_nc()
    in_maps = prep_inputs(x, w_attn, w_proj)
    return run_bass_kernel_spmd(
        nc, in_maps, list(range(N_CORES)), trace=True,
        trace_cores=trace_cores if trace_cores is not None else [0])

